# revision 17
# baseline (speedup 1.0000x reference)
"""Extended S5 SSM on 8 Trainium2 NeuronCores (Bass/Tile).

Sequence-parallel: L sharded across 8 cores, feature-on-partition layout.
Complex diagonal scan via rotation factorization (two real scans per lane
after complex rotation of the input), subchunks of T=512 chained through
per-partition init rotations, cores chained through one 8 KB AllGather of
end-states per scan pass.

v2 layout/engine plan (vs baseline):
  - all data-plane tensors bf16 (params cast on host); PSUM stays f32
  - w (rotated Bu) and y2 are SBUF-resident (y2 overwrites w) -- no spills
  - Delta folded into F on host: p = x_prev @ (F^T Delta), Ep = p @ E^T
  - shift-by-one via access pattern on a padded x1 buffer; cross-core
    carry correction applied in-place with m^t cos/sin tables
  - Act engine does PSUM->SBUF bf16 copies; DMA issue alternates SP/Act
  - rotations on V (bf16 2x) + G split; scans split V/G
"""

import sys

import numpy as np

for _p in ("/opt/trn_rl_repo", "/root/.axon_site/_ro/trn_rl_repo"):
    if _p not in sys.path:
        sys.path.append(_p)

try:
    import ml_dtypes
except ImportError:
    ml_dtypes = None

# ---- problem geometry (hardcoded; harness contract) ----
L, H, P, R = 16384, 1024, 1024, 512
NCORES = 8

CFG_FULL = dict(L=16384, T=512)

_PROG_CACHE = {}


# ======================================================================
# device program
# ======================================================================

def _emit(nc, tc, io, cfg):
    import concourse.mybir as mybir

    f32 = mybir.dt.float32
    bf16 = mybir.dt.bfloat16
    OP = mybir.AluOpType

    T = cfg["T"]
    S = cfg["L"] // NCORES
    NSUB = S // T          # 4 subchunks of T per core
    KH = H // 128          # 8
    PTP = P // 128         # 8
    KR = R // 128          # 4
    HT = H // 128          # 8
    RT = R // 128          # 4

    V = nc.vector
    G = nc.gpsimd
    A = nc.scalar

    _dma_i = [0]

    def dma(dst, src):
        # alternate DMA issue between the SP and Act HWDGE queues
        eng = nc.sync if (_dma_i[0] % 2 == 0) else nc.scalar
        _dma_i[0] += 1
        return eng.dma_start(dst, src)

    with (
        tc.tile_pool(name="glue", bufs=1) as glue,
        tc.tile_pool(name="tabs", bufs=1) as tabs,
        tc.tile_pool(name="wper", bufs=1) as wper,
        tc.tile_pool(name="x1p", bufs=1) as x1p,
        tc.tile_pool(name="dram", bufs=1, space="DRAM") as dram,
    ):
        # ---------- persistent tables (loaded after B so PE starts sooner) ----------
        ctab_t, stab_t = [], []
        mvec_t, cosT_t, sinT_t, cosT1_t, sinT1_t = [], [], [], [], []
        coefre_t, coefim_t, zcol_t, lpr_t, lpi_t = [], [], [], [], []

        def load_tables():
            for pt in range(PTP):
                r0 = pt * 128
                ct = tabs.tile([128, T], bf16, name=f"ctab{pt}", tag=f"ctab{pt}")
                dma(ct[:], io["ctab"].ap()[r0 : r0 + 128, :])
                ctab_t.append(ct)
                st = tabs.tile([128, T], bf16, name=f"stab{pt}", tag=f"stab{pt}")
                dma(st[:], io["stab"].ap()[r0 : r0 + 128, :])
                stab_t.append(st)
                # [mvec cosT sinT cosT1 sinT1 coefre(8) coefim(8) zcol lpr(4) lpi(4)]
                gt = tabs.tile([128, 30], f32, name=f"gtab{pt}", tag=f"gtab{pt}")
                dma(gt[:], io["gtab"].ap()[r0 : r0 + 128, :])
                mvec_t.append(gt[:, 0:1])
                cosT_t.append(gt[:, 1:2])
                sinT_t.append(gt[:, 2:3])
                cosT1_t.append(gt[:, 3:4])
                sinT1_t.append(gt[:, 4:5])
                coefre_t.append(gt[:, 5:13])
                coefim_t.append(gt[:, 13:21])
                zcol_t.append(gt[:, 21:22])
                lpr_t.append(gt[:, 22:26])
                lpi_t.append(gt[:, 26:30])

        # ---------- persistent state ----------
        # w (rotated Bu input), reused as y2 storage in phase 2
        wre_t = [wper.tile([128, S], bf16, name=f"wre{pt}", tag=f"wre{pt}") for pt in range(PTP)]
        wim_t = [wper.tile([128, S], bf16, name=f"wim{pt}", tag=f"wim{pt}") for pt in range(PTP)]
        # x1 (real part of pass-1 states), 1 extra leading col for the shift
        x1u_t = [x1p.tile([128, S + 2], bf16, name=f"x1u{pt}", tag=f"x1u{pt}") for pt in range(PTP)]

        i1re = [glue.tile([128, 1], f32, name=f"i1re{pt}", tag=f"i1re{pt}") for pt in range(PTP)]
        i1im = [glue.tile([128, 1], f32, name=f"i1im{pt}", tag=f"i1im{pt}") for pt in range(PTP)]
        i2re = [glue.tile([128, 1], f32, name=f"i2re{pt}", tag=f"i2re{pt}") for pt in range(PTP)]
        i2im = [glue.tile([128, 1], f32, name=f"i2im{pt}", tag=f"i2im{pt}") for pt in range(PTP)]
        gre_t = [[glue.tile([128, 1], f32, name=f"g{e}re{pt}", tag=f"g{e}re{pt}") for pt in range(PTP)] for e in range(2)]
        gim_t = [[glue.tile([128, 1], f32, name=f"g{e}im{pt}", tag=f"g{e}im{pt}") for pt in range(PTP)] for e in range(2)]
        Gre_t = [[glue.tile([128, NSUB], f32, name=f"G{e}re{pt}", tag=f"G{e}re{pt}") for pt in range(PTP)] for e in range(2)]
        Gim_t = [[glue.tile([128, NSUB], f32, name=f"G{e}im{pt}", tag=f"G{e}im{pt}") for pt in range(PTP)] for e in range(2)]
        nGim_t = [[glue.tile([128, NSUB], f32, name=f"nG{e}im{pt}", tag=f"nG{e}im{pt}") for pt in range(PTP)] for e in range(2)]

        xe_in = [dram.tile([P, 2], f32, name=f"xe_in{e}", tag=f"xe_in{e}") for e in range(2)]
        xe_out = [
            dram.tile([NCORES * P, 2], f32, addr_space="Shared", name=f"xe_out{e}", tag=f"xe_out{e}")
            for e in range(2)
        ]

        def subchunk_glue(y_re_ap, y_im_ap, ire, iim, pt, sc_pool):
            # init_{tau+1} = e^{i T th} * y_last  (per-partition rotation); on V
            yr = y_re_ap[:, T - 1 : T]
            yi = y_im_ap[:, T - 1 : T]
            b = sc_pool.tile([128, 1], f32, tag="gb", bufs=4)
            d = sc_pool.tile([128, 1], f32, tag="gd", bufs=4)
            G.tensor_scalar_mul(b[:], yi, sinT_t[pt])
            G.tensor_scalar_mul(d[:], yi, cosT_t[pt])
            V.scalar_tensor_tensor(ire[:], yr, cosT_t[pt], b[:], op0=OP.mult, op1=OP.subtract)
            V.scalar_tensor_tensor(iim[:], yr, sinT_t[pt], d[:], op0=OP.mult, op1=OP.add)

        def end_state(y_re_ap, y_im_ap, pt, sc_pool, exi):
            # x_end = e^{i (T-1) th} * y_last -> pack (re,im), stage to DRAM
            yr = y_re_ap[:, T - 1 : T]
            yi = y_im_ap[:, T - 1 : T]
            b = sc_pool.tile([128, 1], f32, tag="eb", bufs=4)
            d = sc_pool.tile([128, 1], f32, tag="ed", bufs=4)
            G.tensor_scalar_mul(b[:], yi, sinT1_t[pt])
            G.tensor_scalar_mul(d[:], yi, cosT1_t[pt])
            pk = sc_pool.tile([128, 2], f32, tag="epk", bufs=4)
            V.scalar_tensor_tensor(pk[:, 0:1], yr, cosT1_t[pt], b[:], op0=OP.mult, op1=OP.subtract)
            V.scalar_tensor_tensor(pk[:, 1:2], yr, sinT1_t[pt], d[:], op0=OP.mult, op1=OP.add)
            dma(xe_in[exi][pt * 128 : (pt + 1) * 128, :], pk[:])

        def exchange(exi):
            nc.gpsimd.collective_compute(
                "AllGather",
                mybir.AluOpType.bypass,
                replica_groups=[list(range(NCORES))],
                ins=[xe_in[exi].opt()],
                outs=[xe_out[exi].opt()],
            )

        def exchange_post(exi):
            # combine peer end-states into this core's carry g, and the
            # per-subchunk G_tau = lam^{tau*T+1} * g
            src = xe_out[exi].rearrange("(r q) c -> q r c", r=NCORES)
            for pt in range(PTP):
                xg = glue.tile([128, NCORES, 2], f32, tag="xg", bufs=4)
                dma(xg[:], src[pt * 128 : (pt + 1) * 128, :, :])
                xer = xg[:, :, 0]
                xei = xg[:, :, 1]
                m1 = glue.tile([128, NCORES], f32, tag="m1", bufs=2)
                m2 = glue.tile([128, NCORES], f32, tag="m2", bufs=2)
                m3 = glue.tile([128, NCORES], f32, tag="m3", bufs=2)
                G.tensor_tensor(m1[:], coefre_t[pt], xer, op=OP.mult)
                G.tensor_tensor(m2[:], coefim_t[pt], xei, op=OP.mult)
                V.tensor_tensor(m3[:], m1[:], m2[:], op=OP.subtract)
                V.tensor_reduce(gre_t[exi][pt][:], m3[:], axis=mybir.AxisListType.X, op=OP.add)
                m4 = glue.tile([128, NCORES], f32, tag="m4", bufs=2)
                m5 = glue.tile([128, NCORES], f32, tag="m5", bufs=2)
                m6 = glue.tile([128, NCORES], f32, tag="m6", bufs=2)
                G.tensor_tensor(m4[:], coefre_t[pt], xei, op=OP.mult)
                G.tensor_tensor(m5[:], coefim_t[pt], xer, op=OP.mult)
                V.tensor_tensor(m6[:], m4[:], m5[:], op=OP.add)
                V.tensor_reduce(gim_t[exi][pt][:], m6[:], axis=mybir.AxisListType.X, op=OP.add)
                a = glue.tile([128, NSUB], f32, tag="Ga", bufs=2)
                b = glue.tile([128, NSUB], f32, tag="Gb", bufs=2)
                G.tensor_scalar_mul(a[:], lpr_t[pt], gre_t[exi][pt][:, 0:1])
                G.tensor_scalar_mul(b[:], lpi_t[pt], gim_t[exi][pt][:, 0:1])
                V.tensor_tensor(Gre_t[exi][pt][:], a[:], b[:], op=OP.subtract)
                a2 = glue.tile([128, NSUB], f32, tag="Ga2", bufs=2)
                b2 = glue.tile([128, NSUB], f32, tag="Gb2", bufs=2)
                G.tensor_scalar_mul(a2[:], lpr_t[pt], gim_t[exi][pt][:, 0:1])
                G.tensor_scalar_mul(b2[:], lpi_t[pt], gre_t[exi][pt][:, 0:1])
                V.tensor_tensor(Gim_t[exi][pt][:], a2[:], b2[:], op=OP.add)
                V.tensor_scalar_mul(nGim_t[exi][pt][:], Gim_t[exi][pt][:], -1.0)

        # ==============================================================
        # PHASE 1: Bu matmuls, rotation, local scans, end states
        # ==============================================================
        y3_hold = []  # deferred tau=3 unrotation work (overlaps exchange 0)
        with (
            tc.tile_pool(name="bt", bufs=1) as bt,
            tc.tile_pool(name="p1u", bufs=2) as p1u,
            tc.tile_pool(name="p1", bufs=2) as p1,
            tc.tile_pool(name="p1y", bufs=2) as p1y,
            tc.tile_pool(name="hold3", bufs=1) as hold3,
            tc.tile_pool(name="ps1", bufs=4, space="PSUM") as ps1,
        ):
            ut0 = []
            for k in range(KH):
                t = p1u.tile([128, T], bf16, tag=f"ut{k}")
                dma(t[:], io["uT"].ap()[k * 128 : (k + 1) * 128, 0:T])
                ut0.append(t)
            btre_t, btim_t = [], []
            for k in range(KH):
                t = bt.tile([128, P], bf16, name=f"btre{k}", tag=f"btre{k}")
                dma(t[:], io["BTre"].ap()[k * 128 : (k + 1) * 128, :])
                btre_t.append(t)
            for k in range(KH):
                t = bt.tile([128, P], bf16, name=f"btim{k}", tag=f"btim{k}")
                dma(t[:], io["BTim"].ap()[k * 128 : (k + 1) * 128, :])
                btim_t.append(t)
            load_tables()
            for pt in range(PTP):
                V.memset(i1re[pt][:], 0.0)
                V.memset(i1im[pt][:], 0.0)
                V.memset(i2re[pt][:], 0.0)
                V.memset(i2im[pt][:], 0.0)

            for tau in range(NSUB):
                c0 = tau * T
                cs = slice(c0, c0 + T)
                if tau == 0:
                    ut_t = ut0
                else:
                    ut_t = []
                    for k in range(KH):
                        t = p1u.tile([128, T], bf16, tag=f"ut{k}")
                        dma(t[:], io["uT"].ap()[k * 128 : (k + 1) * 128, cs])
                        ut_t.append(t)
                for pt in range(PTP):
                    pc = slice(pt * 128, (pt + 1) * 128)
                    pre = ps1.tile([128, T], f32, tag="bure")
                    for k in range(KH):
                        nc.tensor.matmul(
                            pre[:], btre_t[k][:, pc], ut_t[k][:],
                            start=(k == 0), stop=(k == KH - 1),
                        )
                    bre = p1.tile([128, T], bf16, tag="bre")
                    A.copy(bre[:], pre[:])
                    pim = ps1.tile([128, T], f32, tag="buim")
                    for k in range(KH):
                        nc.tensor.matmul(
                            pim[:], btim_t[k][:, pc], ut_t[k][:],
                            start=(k == 0), stop=(k == KH - 1),
                        )
                    bim = p1.tile([128, T], bf16, tag="bim")
                    A.copy(bim[:], pim[:])
                    # rotation: wre = c*bre + s*bim ; wim = c*bim - s*bre
                    t1 = p1.tile([128, T], bf16, tag="t1")
                    t2 = p1.tile([128, T], bf16, tag="t2")
                    t3 = p1.tile([128, T], bf16, tag="t3")
                    t4 = p1.tile([128, T], bf16, tag="t4")
                    G.tensor_tensor(t1[:], ctab_t[pt][:], bre[:], op=OP.mult)
                    G.tensor_tensor(t2[:], stab_t[pt][:], bim[:], op=OP.mult)
                    V.tensor_tensor(wre_t[pt][:, cs], t1[:], t2[:], op=OP.add)
                    G.tensor_tensor(t3[:], ctab_t[pt][:], bim[:], op=OP.mult)
                    G.tensor_tensor(t4[:], stab_t[pt][:], bre[:], op=OP.mult)
                    V.tensor_tensor(wim_t[pt][:, cs], t3[:], t4[:], op=OP.subtract)
                    # scans (chained per subchunk)
                    mb = mvec_t[pt].broadcast_to((128, T))
                    defer = tau == NSUB - 1 and pt < 2
                    if defer:
                        yre = hold3.tile([128, T], bf16, name=f"y3r{pt}", tag=f"y3r{pt}")
                        yim = hold3.tile([128, T], bf16, name=f"y3i{pt}", tag=f"y3i{pt}")
                    else:
                        yre = p1y.tile([128, T], bf16, tag="yre")
                        yim = p1y.tile([128, T], bf16, tag="yim")
                    V.tensor_tensor_scan(
                        yre[:], mb, wre_t[pt][:, cs], i1re[pt][:, 0:1],
                        op0=OP.mult, op1=OP.add,
                    )
                    V.tensor_tensor_scan(
                        yim[:], mb, wim_t[pt][:, cs], i1im[pt][:, 0:1],
                        op0=OP.mult, op1=OP.add,
                    )
                    if tau == NSUB - 1:
                        end_state(yre[:], yim[:], pt, p1, 0)
                        if defer:
                            y3_hold.append((pt, yre, yim))
                    else:
                        subchunk_glue(yre[:], yim[:], i1re[pt], i1im[pt], pt, p1)
                    if not (tau == NSUB - 1 and defer):
                        # x1u unrotation: x1(t) = c*yre - s*yim
                        t5 = p1.tile([128, T], bf16, tag="t5")
                        t6 = p1.tile([128, T], bf16, tag="t6")
                        G.tensor_tensor(t5[:], ctab_t[pt][:], yre[:], op=OP.mult)
                        G.tensor_tensor(t6[:], stab_t[pt][:], yim[:], op=OP.mult)
                        V.tensor_tensor(
                            x1u_t[pt][:, 1 + c0 : 1 + c0 + T], t5[:], t6[:], op=OP.subtract
                        )

            # ---- carry exchange 0 (collective overlaps deferred work below) ----
            exchange(0)
            for pt, yre, yim in y3_hold:
                c0 = (NSUB - 1) * T
                t5 = p1.tile([128, T], bf16, tag="t5")
                t6 = p1.tile([128, T], bf16, tag="t6")
                G.tensor_tensor(t5[:], ctab_t[pt][:], yre[:], op=OP.mult)
                G.tensor_tensor(t6[:], stab_t[pt][:], yim[:], op=OP.mult)
                V.tensor_tensor(
                    x1u_t[pt][:, 1 + c0 : 1 + c0 + T], t5[:], t6[:], op=OP.subtract
                )
            exchange_post(0)

        # ==============================================================
        # PHASE 2: in-place carry correction on x1, low-rank path, scan 2
        # ==============================================================
        with (
            tc.tile_pool(name="fde", bufs=1) as fde,
            tc.tile_pool(name="p2", bufs=2) as p2,
            tc.tile_pool(name="ps2", bufs=4, space="PSUM") as ps2,
        ):
            # m^t cos(t th) / m^t sin(t th) (phase-2 frame) and m^t (phase 3)
            mctab_t, mstab_t, mptab_t = [], [], []
            for pt in range(PTP):
                r0 = pt * 128
                t = fde.tile([128, T], bf16, name=f"mctab{pt}", tag=f"mctab{pt}")
                dma(t[:], io["mctab"].ap()[r0 : r0 + 128, :])
                mctab_t.append(t)
                t = fde.tile([128, T], bf16, name=f"mstab{pt}", tag=f"mstab{pt}")
                dma(t[:], io["mstab"].ap()[r0 : r0 + 128, :])
                mstab_t.append(t)
                t = tabs.tile([128, T], bf16, name=f"mptab{pt}", tag=f"mptab{pt}")
                dma(t[:], io["mptab"].ap()[r0 : r0 + 128, :])
                mptab_t.append(t)
            f2_t, et_t = [], []
            for k in range(PTP):
                t = fde.tile([128, R], bf16, name=f"f2{k}", tag=f"f2{k}")
                dma(t[:], io["F2T"].ap()[k * 128 : (k + 1) * 128, :])
                f2_t.append(t)
            for k in range(KR):
                t = fde.tile([128, P], bf16, name=f"et{k}", tag=f"et{k}")
                dma(t[:], io["ET"].ap()[k * 128 : (k + 1) * 128, :])
                et_t.append(t)

            for tau in range(NSUB):
                c0 = tau * T
                cs = slice(c0, c0 + T)
                # x1_corr(t) = x1_local(t) + Gre_tau*mc(t') - Gim_tau*ms(t')
                # in place per subchunk; boundary col0 = x_corr(-1) = gre
                for pt in range(PTP):
                    xs = x1u_t[pt][:, 1 + c0 : 1 + c0 + T]
                    V.scalar_tensor_tensor(
                        xs, mctab_t[pt][:], Gre_t[0][pt][:, tau : tau + 1], xs,
                        op0=OP.mult, op1=OP.add,
                    )
                    gt2 = p2.tile([128, T], bf16, tag="gt2")
                    G.tensor_scalar_mul(gt2[:], mstab_t[pt][:], nGim_t[0][pt][:, tau : tau + 1])
                    G.tensor_tensor(xs, xs, gt2[:], op=OP.add)
                    if tau == 0:
                        V.tensor_copy(x1u_t[pt][:, 0:1], gre_t[0][pt][:])
                # p = x_prev @ F2  (shifted view: buf cols [c0, c0+T))
                p_sb = []
                for rt in range(RT):
                    pp = ps2.tile([128, T], f32, tag="pp")
                    rc = slice(rt * 128, (rt + 1) * 128)
                    for k in range(PTP):
                        nc.tensor.matmul(
                            pp[:], f2_t[k][:, rc], x1u_t[k][:, c0 : c0 + T],
                            start=(k == 0), stop=(k == PTP - 1),
                        )
                    ps_ = p2.tile([128, T], bf16, tag="psb", bufs=5)
                    A.copy(ps_[:], pp[:])
                    p_sb.append(ps_)
                # Ep + w2 + scan2 per pt
                for pt in range(PTP):
                    pc = slice(pt * 128, (pt + 1) * 128)
                    epp = ps2.tile([128, T], f32, tag="ep")
                    for k in range(KR):
                        nc.tensor.matmul(
                            epp[:], et_t[k][:, pc], p_sb[k][:],
                            start=(k == 0), stop=(k == KR - 1),
                        )
                    ep_sb = p2.tile([128, T], bf16, tag="ep_sb")
                    A.copy(ep_sb[:], epp[:])
                    ta = p2.tile([128, T], bf16, tag="ta")
                    tb = p2.tile([128, T], bf16, tag="tb")
                    w2r = p2.tile([128, T], bf16, tag="w2r")
                    w2i = p2.tile([128, T], bf16, tag="w2i")
                    G.tensor_tensor(ta[:], ctab_t[pt][:], ep_sb[:], op=OP.mult)
                    V.tensor_tensor(w2r[:], wre_t[pt][:, cs], ta[:], op=OP.add)
                    G.tensor_tensor(tb[:], stab_t[pt][:], ep_sb[:], op=OP.mult)
                    G.tensor_tensor(w2i[:], wim_t[pt][:, cs], tb[:], op=OP.subtract)
                    if tau == 0:
                        V.tensor_tensor(w2r[:, 0:1], w2r[:, 0:1], zcol_t[pt], op=OP.mult)
                        G.tensor_tensor(w2i[:, 0:1], w2i[:, 0:1], zcol_t[pt], op=OP.mult)
                    # scan outputs overwrite w (consumed above)
                    mb = mvec_t[pt].broadcast_to((128, T))
                    V.tensor_tensor_scan(
                        wre_t[pt][:, cs], mb, w2r[:], i2re[pt][:, 0:1],
                        op0=OP.mult, op1=OP.add,
                    )
                    V.tensor_tensor_scan(
                        wim_t[pt][:, cs], mb, w2i[:], i2im[pt][:, 0:1],
                        op0=OP.mult, op1=OP.add,
                    )
                    if tau == NSUB - 1:
                        end_state(wre_t[pt][:, cs], wim_t[pt][:, cs], pt, p2, 1)
                    else:
                        subchunk_glue(wre_t[pt][:, cs], wim_t[pt][:, cs], i2re[pt], i2im[pt], pt, p2)

        # ==============================================================
        # PHASE 3: y2 carry correction, unrotation, C projection, output
        # ==============================================================
        with (
            tc.tile_pool(name="cpar", bufs=1) as cpar,
            tc.tile_pool(name="p3", bufs=2) as p3,
            tc.tile_pool(name="ps3", bufs=4, space="PSUM") as ps3,
        ):
            cre_t, nci_t = [], []
            for k in range(PTP):
                t = cpar.tile([128, H], bf16, name=f"cre{k}", tag=f"cre{k}")
                dma(t[:], io["CreT"].ap()[k * 128 : (k + 1) * 128, :])
                cre_t.append(t)
                t = cpar.tile([128, H], bf16, name=f"nci{k}", tag=f"nci{k}")
                dma(t[:], io["nCimT"].ap()[k * 128 : (k + 1) * 128, :])
                nci_t.append(t)
            dd_t = []
            for hb in range(HT):
                t = cpar.tile([128, 128], bf16, name=f"dd{hb}", tag=f"dd{hb}")
                dma(t[:], io["Ddiag"].ap()[hb * 128 : (hb + 1) * 128, :])
                dd_t.append(t)

            exchange(1)
            exchange_post(1)

            url_pend = [None] * HT
            for hb in range(HT):
                # prefetch tau=0 u-slices during the collective
                url = p3.tile([128, T], bf16, tag="url", bufs=9, name=f"url_p_{hb}")
                dma(url[:], io["uT"].ap()[hb * 128 : (hb + 1) * 128, 0:T])
                url_pend[hb] = url
            for tau in range(NSUB):
                c0 = tau * T
                cs = slice(c0, c0 + T)
                url_t = url_pend
                if tau + 1 < NSUB:
                    url_pend = [None] * HT
                    for hb in range(HT):
                        url = p3.tile([128, T], bf16, tag="url", bufs=9, name=f"url_{tau}_{hb}")
                        dma(url[:], io["uT"].ap()[hb * 128 : (hb + 1) * 128, c0 + T : c0 + 2 * T])
                        url_pend[hb] = url
                op_t = [
                    ps3.tile([128, T], f32, tag="o", bufs=8, name=f"o_{tau}_{hb}")
                    for hb in range(HT)
                ]
                for pt in range(PTP):
                    # in-place carry correction on y2 (y-frame)
                    V.scalar_tensor_tensor(
                        wre_t[pt][:, cs], mptab_t[pt][:], Gre_t[1][pt][:, tau : tau + 1],
                        wre_t[pt][:, cs], op0=OP.mult, op1=OP.add,
                    )
                    V.scalar_tensor_tensor(
                        wim_t[pt][:, cs], mptab_t[pt][:], Gim_t[1][pt][:, tau : tau + 1],
                        wim_t[pt][:, cs], op0=OP.mult, op1=OP.add,
                    )
                    # unrotation: xre2 = c*y2r - s*y2i ; xim2 = s*y2r + c*y2i
                    u1 = p3.tile([128, T], bf16, tag="u1")
                    u2 = p3.tile([128, T], bf16, tag="u2")
                    u3 = p3.tile([128, T], bf16, tag="u3")
                    u4 = p3.tile([128, T], bf16, tag="u4")
                    xr = p3.tile([128, T], bf16, tag="xr", bufs=3)
                    xi = p3.tile([128, T], bf16, tag="xi", bufs=3)
                    G.tensor_tensor(u1[:], ctab_t[pt][:], wre_t[pt][:, cs], op=OP.mult)
                    G.tensor_tensor(u2[:], stab_t[pt][:], wim_t[pt][:, cs], op=OP.mult)
                    V.tensor_tensor(xr[:], u1[:], u2[:], op=OP.subtract)
                    G.tensor_tensor(u3[:], stab_t[pt][:], wre_t[pt][:, cs], op=OP.mult)
                    G.tensor_tensor(u4[:], ctab_t[pt][:], wim_t[pt][:, cs], op=OP.mult)
                    V.tensor_tensor(xi[:], u3[:], u4[:], op=OP.add)
                    for hb in range(HT):
                        mc = slice(hb * 128, (hb + 1) * 128)
                        nc.tensor.matmul(
                            op_t[hb][:], cre_t[pt][:, mc], xr[:],
                            start=(pt == 0), stop=False,
                        )
                        nc.tensor.matmul(
                            op_t[hb][:], nci_t[pt][:, mc], xi[:],
                            start=False, stop=False,
                        )
                for hb in range(HT):
                    mc = slice(hb * 128, (hb + 1) * 128)
                    nc.tensor.matmul(op_t[hb][:], dd_t[hb][:], url_t[hb][:], start=False, stop=True)
                    osb = p3.tile([128, T], bf16, tag="osb")
                    A.copy(osb[:], op_t[hb][:])
                    dma(io["outT"].ap()[mc, cs], osb[:])


def build_program(cfg):
    import concourse.bacc as bacc
    import concourse.mybir as mybir
    import concourse.tile as tile

    f32 = mybir.dt.float32
    bf16 = mybir.dt.bfloat16
    T = cfg["T"]
    S = cfg["L"] // NCORES

    nc = bacc.Bacc(
        "TRN2", target_bir_lowering=False, debug=False, num_devices=NCORES
    )
    io = {}
    ins = [
        ("uT", (H, S), bf16),
        ("BTre", (H, P), bf16), ("BTim", (H, P), bf16),
        ("CreT", (P, H), bf16), ("nCimT", (P, H), bf16),
        ("F2T", (P, R), bf16), ("ET", (R, P), bf16),
        ("Ddiag", (H, 128), bf16),
        ("ctab", (P, T), bf16), ("stab", (P, T), bf16),
        ("mctab", (P, T), bf16), ("mstab", (P, T), bf16), ("mptab", (P, T), bf16),
        ("gtab", (P, 30), f32),
    ]
    for name, shape, dt_ in ins:
        io[name] = nc.dram_tensor(name, list(shape), dt_, kind="ExternalInput")
    io["outT"] = nc.dram_tensor("outT", [H, S], bf16, kind="ExternalOutput")

    with tile.TileContext(nc) as tc:
        _emit(nc, tc, io, cfg)
    nc.compile()
    return nc


# ======================================================================
# host side
# ======================================================================

def make_tables(lam_re, lam_im, cfg):
    T = cfg["T"]
    S = cfg["L"] // NCORES
    NSUB = S // T
    f32 = np.float32
    bf = ml_dtypes.bfloat16
    lam = lam_re.astype(np.float64) + 1j * lam_im.astype(np.float64)
    mag = np.abs(lam)
    th = np.angle(lam)
    k = np.arange(T)
    ctab = np.cos(np.outer(th, k))
    stab = np.sin(np.outer(th, k))
    mptab = mag[:, None] ** k[None, :]
    tabs = dict(
        ctab=ctab.astype(bf), stab=stab.astype(bf),
        mctab=(mptab * ctab).astype(bf), mstab=(mptab * stab).astype(bf),
        mptab=mptab.astype(bf),
    )
    tau = np.arange(NSUB)
    lpow = lam[:, None] ** (tau[None, :] * T + 1)
    gcols = dict(
        mvec=mag.astype(f32),
        cosT=np.cos(T * th).astype(f32),
        sinT=np.sin(T * th).astype(f32),
        cosT1=np.cos((T - 1) * th).astype(f32),
        sinT1=np.sin((T - 1) * th).astype(f32),
        lpr=np.real(lpow).astype(f32), lpi=np.imag(lpow).astype(f32),
    )
    coefre = np.zeros((NCORES, P, NCORES), f32)
    coefim = np.zeros((NCORES, P, NCORES), f32)
    for m in range(NCORES):
        for j in range(m):
            v = lam ** (S * (m - 1 - j))
            coefre[m, :, j] = np.real(v)
            coefim[m, :, j] = np.imag(v)
    return tabs, gcols, coefre, coefim


def _block_diag_D(D):
    # (H,) -> (H, 128): row h holds D[h] at column h%128 (per-128 diag blocks)
    Hn = D.shape[0]
    out = np.zeros((Hn, 128), np.float32)
    out[np.arange(Hn), np.arange(Hn) % 128] = D
    return out


def make_in_maps(inputs, cfg):
    f32 = np.float32
    bf = ml_dtypes.bfloat16
    Lc = cfg["L"]
    S = Lc // NCORES
    u = np.ascontiguousarray(np.asarray(inputs["input_sequence"], f32)[:Lc])
    tabs, gcols, coefre, coefim = make_tables(
        np.asarray(inputs["Lambda_re"]), np.asarray(inputs["Lambda_im"]), cfg
    )
    F2 = (np.asarray(inputs["F"], np.float64).T @ np.asarray(inputs["Delta"], np.float64))
    shared = dict(
        BTre=np.ascontiguousarray(np.asarray(inputs["B_re"], f32).T).astype(bf),
        BTim=np.ascontiguousarray(np.asarray(inputs["B_im"], f32).T).astype(bf),
        CreT=np.ascontiguousarray(np.asarray(inputs["C_re"], f32).T).astype(bf),
        nCimT=np.ascontiguousarray(-np.asarray(inputs["C_im"], f32).T).astype(bf),
        F2T=np.ascontiguousarray(F2).astype(bf),
        ET=np.ascontiguousarray(np.asarray(inputs["E"], f32).T).astype(bf),
        Ddiag=np.ascontiguousarray(_block_diag_D(np.asarray(inputs["D"], f32))).astype(bf),
        **tabs,
    )
    in_maps = []
    for m in range(NCORES):
        zcol = np.ones((P, 1), f32)
        if m == 0:
            zcol[:, 0] = 0.0
        gtab = np.concatenate(
            [
                gcols["mvec"][:, None], gcols["cosT"][:, None], gcols["sinT"][:, None],
                gcols["cosT1"][:, None], gcols["sinT1"][:, None],
                coefre[m], coefim[m], zcol,
                gcols["lpr"], gcols["lpi"],
            ],
            axis=1,
        ).astype(f32)
        im = dict(shared)
        im["uT"] = np.ascontiguousarray(u[m * S : (m + 1) * S, :].T).astype(bf)
        im["gtab"] = np.ascontiguousarray(gtab)
        in_maps.append(im)
    return in_maps


def assemble_output(results, cfg):
    Lc = cfg["L"]
    S = Lc // NCORES
    out = np.empty((Lc, H), np.float32)
    for m in range(NCORES):
        out[m * S : (m + 1) * S, :] = results[m]["outT"].T.astype(np.float32)
    out[0, :] = 0.0
    return out


def get_program(cfg_key="full"):
    if cfg_key not in _PROG_CACHE:
        _PROG_CACHE[cfg_key] = build_program(CFG_FULL)
    return _PROG_CACHE[cfg_key]


def run(inputs, trace=False, **kw):
    from concourse import bass_utils

    nc = get_program()
    in_maps = make_in_maps(inputs, CFG_FULL)
    res = bass_utils.run_bass_kernel_spmd(
        nc, in_maps, core_ids=list(range(NCORES)), trace=trace, **kw
    )
    return assemble_output(res.results, CFG_FULL), res


def kernel(**inputs):
    out, _ = run(inputs)
    return out


# revision 30
# speedup vs baseline: 1.0474x; 1.0474x over previous
"""Extended S5 SSM on 8 Trainium2 NeuronCores (Bass/Tile).

Sequence-parallel: L sharded across 8 cores, feature-on-partition layout.
Complex diagonal scan via rotation factorization (two real scans per lane
after complex rotation of the input), subchunks of T=512 chained through
per-partition init rotations, cores chained through one 8 KB AllGather of
end-states per scan pass.

v2 layout/engine plan (vs baseline):
  - all data-plane tensors bf16 (params cast on host); PSUM stays f32
  - w (rotated Bu) and y2 are SBUF-resident (y2 overwrites w) -- no spills
  - Delta folded into F on host: p = x_prev @ (F^T Delta), Ep = p @ E^T
  - shift-by-one via access pattern on a padded x1 buffer; cross-core
    carry correction applied in-place with m^t cos/sin tables
  - Act engine does PSUM->SBUF bf16 copies; DMA issue alternates SP/Act
  - rotations on V (bf16 2x) + G split; scans split V/G
"""

import sys

import numpy as np

for _p in ("/opt/trn_rl_repo", "/root/.axon_site/_ro/trn_rl_repo"):
    if _p not in sys.path:
        sys.path.append(_p)

try:
    import ml_dtypes
except ImportError:
    ml_dtypes = None

# ---- problem geometry (hardcoded; harness contract) ----
L, H, P, R = 16384, 1024, 1024, 512
NCORES = 8

CFG_FULL = dict(L=16384, T=512)

_PROG_CACHE = {}


# ======================================================================
# device program
# ======================================================================

def _emit(nc, tc, io, cfg):
    import concourse.mybir as mybir

    f32 = mybir.dt.float32
    bf16 = mybir.dt.bfloat16
    OP = mybir.AluOpType

    T = cfg["T"]
    S = cfg["L"] // NCORES
    NSUB = S // T          # 4 subchunks of T per core
    KH = H // 128          # 8
    PTP = P // 128         # 8
    KR = R // 128          # 4
    HT = H // 128          # 8
    RT = R // 128          # 4

    V = nc.vector
    G = nc.gpsimd
    A = nc.scalar

    _dma_i = [0]

    def dma(dst, src):
        # alternate DMA issue between the SP and Act HWDGE queues
        eng = nc.sync if (_dma_i[0] % 2 == 0) else nc.scalar
        _dma_i[0] += 1
        return eng.dma_start(dst, src)

    with (
        tc.tile_pool(name="glue", bufs=1) as glue,
        tc.tile_pool(name="tabs", bufs=1) as tabs,
        tc.tile_pool(name="wper", bufs=1) as wper,
        tc.tile_pool(name="x1p", bufs=1) as x1p,
        tc.tile_pool(name="dram", bufs=1, space="DRAM") as dram,
    ):
        # ---------- persistent tables (loaded after B so PE starts sooner) ----------
        ctab_t, stab_t = [], []
        mvec_t, cosT_t, sinT_t, cosT1_t, sinT1_t = [], [], [], [], []
        coefre_t, coefim_t, zcol_t, lpr_t, lpi_t = [], [], [], [], []

        def load_tables():
            for pt in range(PTP):
                r0 = pt * 128
                ct = tabs.tile([128, T], bf16, name=f"ctab{pt}", tag=f"ctab{pt}")
                dma(ct[:], io["ctab"].ap()[r0 : r0 + 128, :])
                ctab_t.append(ct)
                st = tabs.tile([128, T], bf16, name=f"stab{pt}", tag=f"stab{pt}")
                dma(st[:], io["stab"].ap()[r0 : r0 + 128, :])
                stab_t.append(st)
                # [mvec cosT sinT cosT1 sinT1 coefre(8) coefim(8) zcol lpr(4) lpi(4)]
                gt = tabs.tile([128, 30], f32, name=f"gtab{pt}", tag=f"gtab{pt}")
                dma(gt[:], io["gtab"].ap()[r0 : r0 + 128, :])
                mvec_t.append(gt[:, 0:1])
                cosT_t.append(gt[:, 1:2])
                sinT_t.append(gt[:, 2:3])
                cosT1_t.append(gt[:, 3:4])
                sinT1_t.append(gt[:, 4:5])
                coefre_t.append(gt[:, 5:13])
                coefim_t.append(gt[:, 13:21])
                zcol_t.append(gt[:, 21:22])
                lpr_t.append(gt[:, 22:26])
                lpi_t.append(gt[:, 26:30])

        # ---------- persistent state ----------
        # w (rotated Bu input), reused as y2 storage in phase 2
        wre_t = [wper.tile([128, S], bf16, name=f"wre{pt}", tag=f"wre{pt}") for pt in range(PTP)]
        wim_t = [wper.tile([128, S], bf16, name=f"wim{pt}", tag=f"wim{pt}") for pt in range(PTP)]
        # x1 (real part of pass-1 states), 1 extra leading col for the shift
        x1u_t = [x1p.tile([128, S + 2], bf16, name=f"x1u{pt}", tag=f"x1u{pt}") for pt in range(PTP)]

        i1re = [glue.tile([128, 1], f32, name=f"i1re{pt}", tag=f"i1re{pt}") for pt in range(PTP)]
        i1im = [glue.tile([128, 1], f32, name=f"i1im{pt}", tag=f"i1im{pt}") for pt in range(PTP)]
        i2re = [glue.tile([128, 1], f32, name=f"i2re{pt}", tag=f"i2re{pt}") for pt in range(PTP)]
        i2im = [glue.tile([128, 1], f32, name=f"i2im{pt}", tag=f"i2im{pt}") for pt in range(PTP)]
        gre_t = [[glue.tile([128, 1], f32, name=f"g{e}re{pt}", tag=f"g{e}re{pt}") for pt in range(PTP)] for e in range(2)]
        gim_t = [[glue.tile([128, 1], f32, name=f"g{e}im{pt}", tag=f"g{e}im{pt}") for pt in range(PTP)] for e in range(2)]
        Gre_t = [[glue.tile([128, NSUB], f32, name=f"G{e}re{pt}", tag=f"G{e}re{pt}") for pt in range(PTP)] for e in range(2)]
        Gim_t = [[glue.tile([128, NSUB], f32, name=f"G{e}im{pt}", tag=f"G{e}im{pt}") for pt in range(PTP)] for e in range(2)]
        nGim_t = [[glue.tile([128, NSUB], f32, name=f"nG{e}im{pt}", tag=f"nG{e}im{pt}") for pt in range(PTP)] for e in range(2)]

        xe_in = [dram.tile([P, 2], bf16, name=f"xe_in{e}", tag=f"xe_in{e}") for e in range(2)]
        xe_out = [
            dram.tile([NCORES * P, 2], bf16, addr_space="Shared", name=f"xe_out{e}", tag=f"xe_out{e}")
            for e in range(2)
        ]

        def subchunk_glue(y_re_ap, y_im_ap, ire, iim, pt, sc_pool):
            # init_{tau+1} = e^{i T th} * y_last  (per-partition rotation); on V
            yr = y_re_ap[:, T - 1 : T]
            yi = y_im_ap[:, T - 1 : T]
            b = sc_pool.tile([128, 1], f32, tag="gb", bufs=4)
            d = sc_pool.tile([128, 1], f32, tag="gd", bufs=4)
            G.tensor_scalar_mul(b[:], yi, sinT_t[pt])
            G.tensor_scalar_mul(d[:], yi, cosT_t[pt])
            V.scalar_tensor_tensor(ire[:], yr, cosT_t[pt], b[:], op0=OP.mult, op1=OP.subtract)
            V.scalar_tensor_tensor(iim[:], yr, sinT_t[pt], d[:], op0=OP.mult, op1=OP.add)

        def end_state(y_re_ap, y_im_ap, pt, sc_pool, exi):
            # x_end = e^{i (T-1) th} * y_last -> pack (re,im), stage to DRAM
            yr = y_re_ap[:, T - 1 : T]
            yi = y_im_ap[:, T - 1 : T]
            b = sc_pool.tile([128, 1], f32, tag="eb", bufs=4)
            d = sc_pool.tile([128, 1], f32, tag="ed", bufs=4)
            G.tensor_scalar_mul(b[:], yi, sinT1_t[pt])
            G.tensor_scalar_mul(d[:], yi, cosT1_t[pt])
            pk = sc_pool.tile([128, 2], bf16, tag="epk", bufs=4)
            V.scalar_tensor_tensor(pk[:, 0:1], yr, cosT1_t[pt], b[:], op0=OP.mult, op1=OP.subtract)
            V.scalar_tensor_tensor(pk[:, 1:2], yr, sinT1_t[pt], d[:], op0=OP.mult, op1=OP.add)
            dma(xe_in[exi][pt * 128 : (pt + 1) * 128, :], pk[:])

        def exchange(exi):
            nc.gpsimd.collective_compute(
                "AllGather",
                mybir.AluOpType.bypass,
                replica_groups=[list(range(NCORES))],
                ins=[xe_in[exi].opt()],
                outs=[xe_out[exi].opt()],
            )

        def exchange_post(exi):
            # combine peer end-states into this core's carry g, and the
            # per-subchunk G_tau = lam^{tau*T+1} * g
            src = xe_out[exi].rearrange("(r q) c -> q r c", r=NCORES)
            for pt in range(PTP):
                xg = glue.tile([128, NCORES, 2], bf16, tag="xg", bufs=4)
                dma(xg[:], src[pt * 128 : (pt + 1) * 128, :, :])
                xer = xg[:, :, 0]
                xei = xg[:, :, 1]
                m1 = glue.tile([128, NCORES], f32, tag="m1", bufs=2)
                m2 = glue.tile([128, NCORES], f32, tag="m2", bufs=2)
                m3 = glue.tile([128, NCORES], f32, tag="m3", bufs=2)
                G.tensor_tensor(m1[:], coefre_t[pt], xer, op=OP.mult)
                G.tensor_tensor(m2[:], coefim_t[pt], xei, op=OP.mult)
                V.tensor_tensor(m3[:], m1[:], m2[:], op=OP.subtract)
                V.tensor_reduce(gre_t[exi][pt][:], m3[:], axis=mybir.AxisListType.X, op=OP.add)
                m4 = glue.tile([128, NCORES], f32, tag="m4", bufs=2)
                m5 = glue.tile([128, NCORES], f32, tag="m5", bufs=2)
                m6 = glue.tile([128, NCORES], f32, tag="m6", bufs=2)
                G.tensor_tensor(m4[:], coefre_t[pt], xei, op=OP.mult)
                G.tensor_tensor(m5[:], coefim_t[pt], xer, op=OP.mult)
                V.tensor_tensor(m6[:], m4[:], m5[:], op=OP.add)
                V.tensor_reduce(gim_t[exi][pt][:], m6[:], axis=mybir.AxisListType.X, op=OP.add)
                a = glue.tile([128, NSUB], f32, tag="Ga", bufs=2)
                b = glue.tile([128, NSUB], f32, tag="Gb", bufs=2)
                G.tensor_scalar_mul(a[:], lpr_t[pt], gre_t[exi][pt][:, 0:1])
                G.tensor_scalar_mul(b[:], lpi_t[pt], gim_t[exi][pt][:, 0:1])
                V.tensor_tensor(Gre_t[exi][pt][:], a[:], b[:], op=OP.subtract)
                a2 = glue.tile([128, NSUB], f32, tag="Ga2", bufs=2)
                b2 = glue.tile([128, NSUB], f32, tag="Gb2", bufs=2)
                G.tensor_scalar_mul(a2[:], lpr_t[pt], gim_t[exi][pt][:, 0:1])
                G.tensor_scalar_mul(b2[:], lpi_t[pt], gre_t[exi][pt][:, 0:1])
                V.tensor_tensor(Gim_t[exi][pt][:], a2[:], b2[:], op=OP.add)
                if exi == 0:
                    V.tensor_scalar_mul(nGim_t[exi][pt][:], Gim_t[exi][pt][:], -1.0)

        # ==============================================================
        # PHASE 1: Bu matmuls, rotation, local scans, end states
        # ==============================================================
        y3_hold = []  # deferred tau=3 unrotation work (overlaps exchange 0)
        with (
            tc.tile_pool(name="bt", bufs=1) as bt,
            tc.tile_pool(name="p1u", bufs=2) as p1u,
            tc.tile_pool(name="p1", bufs=3) as p1,
            tc.tile_pool(name="p1y", bufs=2) as p1y,
            tc.tile_pool(name="hold3", bufs=1) as hold3,
            tc.tile_pool(name="ps1", bufs=4, space="PSUM") as ps1,
        ):
            ut0, btre_t, btim_t = [], [], []
            for k in range(KH):
                t = bt.tile([128, P], bf16, name=f"btre{k}", tag=f"btre{k}")
                dma(t[:], io["BTre"].ap()[k * 128 : (k + 1) * 128, :])
                btre_t.append(t)
                t = p1u.tile([128, T], bf16, tag=f"ut{k}")
                dma(t[:], io["uT"].ap()[k * 128 : (k + 1) * 128, 0:T])
                ut0.append(t)
            for k in range(KH):
                t = bt.tile([128, P], bf16, name=f"btim{k}", tag=f"btim{k}")
                dma(t[:], io["BTim"].ap()[k * 128 : (k + 1) * 128, :])
                btim_t.append(t)
            load_tables()
            for pt in range(PTP):
                V.memset(i1re[pt][:], 0.0)
                V.memset(i1im[pt][:], 0.0)
                V.memset(i2re[pt][:], 0.0)
                V.memset(i2im[pt][:], 0.0)

            for tau in range(NSUB):
                c0 = tau * T
                cs = slice(c0, c0 + T)
                if tau == 0:
                    ut_t = ut0
                else:
                    ut_t = []
                    for k in range(KH):
                        t = p1u.tile([128, T], bf16, tag=f"ut{k}")
                        dma(t[:], io["uT"].ap()[k * 128 : (k + 1) * 128, cs])
                        ut_t.append(t)
                for pt in range(PTP):
                    pc = slice(pt * 128, (pt + 1) * 128)
                    pre = ps1.tile([128, T], f32, tag="bure")
                    for k in range(KH):
                        nc.tensor.matmul(
                            pre[:], btre_t[k][:, pc], ut_t[k][:],
                            start=(k == 0), stop=(k == KH - 1),
                        )
                    bre = p1.tile([128, T], bf16, tag="bre")
                    A.copy(bre[:], pre[:])
                    pim = ps1.tile([128, T], f32, tag="buim")
                    for k in range(KH):
                        nc.tensor.matmul(
                            pim[:], btim_t[k][:, pc], ut_t[k][:],
                            start=(k == 0), stop=(k == KH - 1),
                        )
                    bim = p1.tile([128, T], bf16, tag="bim")
                    A.copy(bim[:], pim[:])
                    # rotation: wre = c*bre + s*bim ; wim = c*bim - s*bre
                    t1 = p1.tile([128, T], bf16, tag="t1")
                    t2 = p1.tile([128, T], bf16, tag="t2")
                    t3 = p1.tile([128, T], bf16, tag="t3")
                    t4 = p1.tile([128, T], bf16, tag="t4")
                    G.tensor_tensor(t1[:], ctab_t[pt][:], bre[:], op=OP.mult)
                    G.tensor_tensor(t2[:], stab_t[pt][:], bim[:], op=OP.mult)
                    V.tensor_tensor(wre_t[pt][:, cs], t1[:], t2[:], op=OP.add)
                    G.tensor_tensor(t3[:], ctab_t[pt][:], bim[:], op=OP.mult)
                    G.tensor_tensor(t4[:], stab_t[pt][:], bre[:], op=OP.mult)
                    V.tensor_tensor(wim_t[pt][:, cs], t3[:], t4[:], op=OP.subtract)
                    # scans (chained per subchunk)
                    mb = mvec_t[pt].broadcast_to((128, T))
                    defer = tau == NSUB - 1 and pt >= PTP - 2
                    if defer:
                        yre = hold3.tile([128, T], bf16, name=f"y3r{pt}", tag=f"y3r{pt}")
                        yim = hold3.tile([128, T], bf16, name=f"y3i{pt}", tag=f"y3i{pt}")
                    else:
                        yre = p1y.tile([128, T], bf16, tag="yre")
                        yim = p1y.tile([128, T], bf16, tag="yim")
                    V.tensor_tensor_scan(
                        yre[:], mb, wre_t[pt][:, cs], i1re[pt][:, 0:1],
                        op0=OP.mult, op1=OP.add,
                    )
                    V.tensor_tensor_scan(
                        yim[:], mb, wim_t[pt][:, cs], i1im[pt][:, 0:1],
                        op0=OP.mult, op1=OP.add,
                    )
                    if tau == NSUB - 1:
                        end_state(yre[:], yim[:], pt, p1, 0)
                        if defer:
                            y3_hold.append((pt, yre, yim))
                    else:
                        subchunk_glue(yre[:], yim[:], i1re[pt], i1im[pt], pt, p1)
                    if not (tau == NSUB - 1 and defer):
                        # x1u unrotation: x1(t) = c*yre - s*yim
                        t5 = p1.tile([128, T], bf16, tag="t5")
                        t6 = p1.tile([128, T], bf16, tag="t6")
                        G.tensor_tensor(t5[:], ctab_t[pt][:], yre[:], op=OP.mult)
                        G.tensor_tensor(t6[:], stab_t[pt][:], yim[:], op=OP.mult)
                        V.tensor_tensor(
                            x1u_t[pt][:, 1 + c0 : 1 + c0 + T], t5[:], t6[:], op=OP.subtract
                        )

            # ---- carry exchange 0 (collective overlaps deferred work below) ----
            exchange(0)
            for pt, yre, yim in y3_hold:
                c0 = (NSUB - 1) * T
                t5 = p1.tile([128, T], bf16, tag="t5")
                t6 = p1.tile([128, T], bf16, tag="t6")
                G.tensor_tensor(t5[:], ctab_t[pt][:], yre[:], op=OP.mult)
                G.tensor_tensor(t6[:], stab_t[pt][:], yim[:], op=OP.mult)
                V.tensor_tensor(
                    x1u_t[pt][:, 1 + c0 : 1 + c0 + T], t5[:], t6[:], op=OP.subtract
                )
            exchange_post(0)

        # ==============================================================
        # PHASE 2: in-place carry correction on x1, low-rank path, scan 2
        # ==============================================================
        with (
            tc.tile_pool(name="fde", bufs=1) as fde,
            tc.tile_pool(name="p2", bufs=3) as p2,
            tc.tile_pool(name="ps2", bufs=4, space="PSUM") as ps2,
        ):
            # m^t cos(t th) / m^t sin(t th) (phase-2 frame) and m^t (phase 3)
            mctab_t, mstab_t, mptab_t = [], [], []
            for pt in range(PTP):
                r0 = pt * 128
                t = fde.tile([128, T], bf16, name=f"mctab{pt}", tag=f"mctab{pt}")
                dma(t[:], io["mctab"].ap()[r0 : r0 + 128, :])
                mctab_t.append(t)
                t = fde.tile([128, T], bf16, name=f"mstab{pt}", tag=f"mstab{pt}")
                dma(t[:], io["mstab"].ap()[r0 : r0 + 128, :])
                mstab_t.append(t)
                t = tabs.tile([128, T], bf16, name=f"mptab{pt}", tag=f"mptab{pt}")
                dma(t[:], io["mptab"].ap()[r0 : r0 + 128, :])
                mptab_t.append(t)
            f2_t, et_t = [], []
            for k in range(PTP):
                t = fde.tile([128, R], bf16, name=f"f2{k}", tag=f"f2{k}")
                dma(t[:], io["F2T"].ap()[k * 128 : (k + 1) * 128, :])
                f2_t.append(t)
            for k in range(KR):
                t = fde.tile([128, P], bf16, name=f"et{k}", tag=f"et{k}")
                dma(t[:], io["ET"].ap()[k * 128 : (k + 1) * 128, :])
                et_t.append(t)

            for tau in range(NSUB):
                c0 = tau * T
                cs = slice(c0, c0 + T)
                # x1_corr(t) = x1_local(t) + Gre_tau*mc(t') - Gim_tau*ms(t')
                # in place per subchunk; boundary col0 = x_corr(-1) = gre
                for pt in range(PTP):
                    xs = x1u_t[pt][:, 1 + c0 : 1 + c0 + T]
                    V.scalar_tensor_tensor(
                        xs, mctab_t[pt][:], Gre_t[0][pt][:, tau : tau + 1], xs,
                        op0=OP.mult, op1=OP.add,
                    )
                    gt2 = p2.tile([128, T], bf16, tag="gt2")
                    G.tensor_scalar_mul(gt2[:], mstab_t[pt][:], nGim_t[0][pt][:, tau : tau + 1])
                    G.tensor_tensor(xs, xs, gt2[:], op=OP.add)
                    if tau == 0:
                        V.tensor_copy(x1u_t[pt][:, 0:1], gre_t[0][pt][:])
                # p = x_prev @ F2  (shifted view: buf cols [c0, c0+T))
                p_sb = []
                for rt in range(RT):
                    pp = ps2.tile([128, T], f32, tag="pp", bufs=6)
                    rc = slice(rt * 128, (rt + 1) * 128)
                    for k in range(PTP):
                        nc.tensor.matmul(
                            pp[:], f2_t[k][:, rc], x1u_t[k][:, c0 : c0 + T],
                            start=(k == 0), stop=(k == PTP - 1),
                        )
                    ps_ = p2.tile([128, T], bf16, tag="psb", bufs=5)
                    A.copy(ps_[:], pp[:])
                    p_sb.append(ps_)
                # Ep + w2 + scan2 per pt
                for pt in range(PTP):
                    pc = slice(pt * 128, (pt + 1) * 128)
                    epp = ps2.tile([128, T], f32, tag="ep", bufs=2)
                    for k in range(KR):
                        nc.tensor.matmul(
                            epp[:], et_t[k][:, pc], p_sb[k][:],
                            start=(k == 0), stop=(k == KR - 1),
                        )
                    ep_sb = p2.tile([128, T], bf16, tag="ep_sb")
                    A.copy(ep_sb[:], epp[:])
                    ta = p2.tile([128, T], bf16, tag="ta")
                    tb = p2.tile([128, T], bf16, tag="tb")
                    w2r = p2.tile([128, T], bf16, tag="w2r")
                    w2i = p2.tile([128, T], bf16, tag="w2i")
                    G.tensor_tensor(ta[:], ctab_t[pt][:], ep_sb[:], op=OP.mult)
                    V.tensor_tensor(w2r[:], wre_t[pt][:, cs], ta[:], op=OP.add)
                    G.tensor_tensor(tb[:], stab_t[pt][:], ep_sb[:], op=OP.mult)
                    G.tensor_tensor(w2i[:], wim_t[pt][:, cs], tb[:], op=OP.subtract)
                    if tau == 0:
                        V.tensor_tensor(w2r[:, 0:1], w2r[:, 0:1], zcol_t[pt], op=OP.mult)
                        G.tensor_tensor(w2i[:, 0:1], w2i[:, 0:1], zcol_t[pt], op=OP.mult)
                    # scan outputs overwrite w (consumed above)
                    mb = mvec_t[pt].broadcast_to((128, T))
                    V.tensor_tensor_scan(
                        wre_t[pt][:, cs], mb, w2r[:], i2re[pt][:, 0:1],
                        op0=OP.mult, op1=OP.add,
                    )
                    V.tensor_tensor_scan(
                        wim_t[pt][:, cs], mb, w2i[:], i2im[pt][:, 0:1],
                        op0=OP.mult, op1=OP.add,
                    )
                    if tau == NSUB - 1:
                        end_state(wre_t[pt][:, cs], wim_t[pt][:, cs], pt, p2, 1)
                    else:
                        subchunk_glue(wre_t[pt][:, cs], wim_t[pt][:, cs], i2re[pt], i2im[pt], pt, p2)

        # ==============================================================
        # PHASE 3: y2 carry correction, unrotation, C projection, output
        # ==============================================================
        with (
            tc.tile_pool(name="cpar", bufs=1) as cpar,
            tc.tile_pool(name="p3", bufs=3) as p3,
            tc.tile_pool(name="ps3", bufs=4, space="PSUM") as ps3,
        ):
            cre_t, nci_t = [], []
            for k in range(PTP):
                t = cpar.tile([128, H], bf16, name=f"cre{k}", tag=f"cre{k}")
                dma(t[:], io["CreT"].ap()[k * 128 : (k + 1) * 128, :])
                cre_t.append(t)
                t = cpar.tile([128, H], bf16, name=f"nci{k}", tag=f"nci{k}")
                dma(t[:], io["nCimT"].ap()[k * 128 : (k + 1) * 128, :])
                nci_t.append(t)
            dd_t = []
            for hb in range(HT):
                t = cpar.tile([128, 128], bf16, name=f"dd{hb}", tag=f"dd{hb}")
                dma(t[:], io["Ddiag"].ap()[hb * 128 : (hb + 1) * 128, :])
                dd_t.append(t)

            # tau=0 u-slices + D-matmul group starters run BEFORE the
            # exchange: PE fills the phase-2 drain / collective window, and
            # the group-start leaves the post-exchange critical path
            url0 = []
            op0_t = []
            for hb in range(HT):
                url = p3.tile([128, T], bf16, tag="url0", bufs=8, name=f"url0_{hb}")
                dma(url[:], io["uT"].ap()[hb * 128 : (hb + 1) * 128, 0:T])
                url0.append(url)
            for hb in range(HT):
                op_ = ps3.tile([128, T], f32, tag="o", bufs=8, name=f"o0_{hb}")
                nc.tensor.matmul(op_[:], dd_t[hb][:], url0[hb][:], start=True, stop=False)
                op0_t.append(op_)

            exchange(1)
            exchange_post(1)
            for tau in range(NSUB):
                c0 = tau * T
                cs = slice(c0, c0 + T)
                if tau == 0:
                    # pt-outer: PE starts after the first pt's unrot instead of
                    # waiting for all 8 (exchange-1 ramp)
                    op_t = op0_t
                    for pt in range(PTP):
                        V.scalar_tensor_tensor(
                            wre_t[pt][:, cs], mptab_t[pt][:], Gre_t[1][pt][:, 0:1],
                            wre_t[pt][:, cs], op0=OP.mult, op1=OP.add,
                        )
                        V.scalar_tensor_tensor(
                            wim_t[pt][:, cs], mptab_t[pt][:], Gim_t[1][pt][:, 0:1],
                            wim_t[pt][:, cs], op0=OP.mult, op1=OP.add,
                        )
                        u1 = p3.tile([128, T], bf16, tag="u1")
                        u2 = p3.tile([128, T], bf16, tag="u2")
                        u3 = p3.tile([128, T], bf16, tag="u3")
                        u4 = p3.tile([128, T], bf16, tag="u4")
                        xr = p3.tile([128, T], bf16, tag="xr", bufs=10)
                        xi = p3.tile([128, T], bf16, tag="xi", bufs=10)
                        G.tensor_tensor(u1[:], ctab_t[pt][:], wre_t[pt][:, cs], op=OP.mult)
                        G.tensor_tensor(u2[:], stab_t[pt][:], wim_t[pt][:, cs], op=OP.mult)
                        V.tensor_tensor(xr[:], u1[:], u2[:], op=OP.subtract)
                        G.tensor_tensor(u3[:], stab_t[pt][:], wre_t[pt][:, cs], op=OP.mult)
                        G.tensor_tensor(u4[:], ctab_t[pt][:], wim_t[pt][:, cs], op=OP.mult)
                        V.tensor_tensor(xi[:], u3[:], u4[:], op=OP.add)
                        for hb in range(HT):
                            mc = slice(hb * 128, (hb + 1) * 128)
                            nc.tensor.matmul(
                                op_t[hb][:], cre_t[pt][:, mc], xr[:],
                                start=False, stop=False,
                            )
                            nc.tensor.matmul(
                                op_t[hb][:], nci_t[pt][:, mc], xi[:],
                                start=False, stop=(pt == PTP - 1),
                            )
                    for hb in range(HT):
                        mc = slice(hb * 128, (hb + 1) * 128)
                        osb = p3.tile([128, T], bf16, tag="osb")
                        A.copy(osb[:], op_t[hb][:])
                        dma(io["outT"].ap()[mc, 0:T], osb[:])
                    continue
                xre2, xim2 = [], []
                for pt in range(PTP):
                    # in-place carry correction on y2 (y-frame)
                    V.scalar_tensor_tensor(
                        wre_t[pt][:, cs], mptab_t[pt][:], Gre_t[1][pt][:, tau : tau + 1],
                        wre_t[pt][:, cs], op0=OP.mult, op1=OP.add,
                    )
                    V.scalar_tensor_tensor(
                        wim_t[pt][:, cs], mptab_t[pt][:], Gim_t[1][pt][:, tau : tau + 1],
                        wim_t[pt][:, cs], op0=OP.mult, op1=OP.add,
                    )
                    # unrotation: xre2 = c*y2r - s*y2i ; xim2 = s*y2r + c*y2i
                    u1 = p3.tile([128, T], bf16, tag="u1")
                    u2 = p3.tile([128, T], bf16, tag="u2")
                    u3 = p3.tile([128, T], bf16, tag="u3")
                    u4 = p3.tile([128, T], bf16, tag="u4")
                    xr = p3.tile([128, T], bf16, tag="xr", bufs=10)
                    xi = p3.tile([128, T], bf16, tag="xi", bufs=10)
                    G.tensor_tensor(u1[:], ctab_t[pt][:], wre_t[pt][:, cs], op=OP.mult)
                    G.tensor_tensor(u2[:], stab_t[pt][:], wim_t[pt][:, cs], op=OP.mult)
                    V.tensor_tensor(xr[:], u1[:], u2[:], op=OP.subtract)
                    G.tensor_tensor(u3[:], stab_t[pt][:], wre_t[pt][:, cs], op=OP.mult)
                    G.tensor_tensor(u4[:], ctab_t[pt][:], wim_t[pt][:, cs], op=OP.mult)
                    V.tensor_tensor(xi[:], u3[:], u4[:], op=OP.add)
                    xre2.append(xr)
                    xim2.append(xi)
                for hb in range(HT):
                    mc = slice(hb * 128, (hb + 1) * 128)
                    url = p3.tile([128, T], bf16, tag="url", bufs=4)
                    dma(url[:], io["uT"].ap()[mc, cs])
                    op_ = ps3.tile([128, T], f32, tag="o", bufs=8)
                    for k in range(PTP):
                        nc.tensor.matmul(
                            op_[:], cre_t[k][:, mc], xre2[k][:],
                            start=(k == 0), stop=False,
                        )
                    for k in range(PTP):
                        nc.tensor.matmul(
                            op_[:], nci_t[k][:, mc], xim2[k][:],
                            start=False, stop=False,
                        )
                    nc.tensor.matmul(op_[:], dd_t[hb][:], url[:], start=False, stop=True)
                    osb = p3.tile([128, T], bf16, tag="osb")
                    A.copy(osb[:], op_[:])
                    dma(io["outT"].ap()[mc, cs], osb[:])


def build_program(cfg):
    import concourse.bacc as bacc
    import concourse.mybir as mybir
    import concourse.tile as tile

    f32 = mybir.dt.float32
    bf16 = mybir.dt.bfloat16
    T = cfg["T"]
    S = cfg["L"] // NCORES

    nc = bacc.Bacc(
        "TRN2", target_bir_lowering=False, debug=False, num_devices=NCORES
    )
    io = {}
    ins = [
        ("uT", (H, S), bf16),
        ("BTre", (H, P), bf16), ("BTim", (H, P), bf16),
        ("CreT", (P, H), bf16), ("nCimT", (P, H), bf16),
        ("F2T", (P, R), bf16), ("ET", (R, P), bf16),
        ("Ddiag", (H, 128), bf16),
        ("ctab", (P, T), bf16), ("stab", (P, T), bf16),
        ("mctab", (P, T), bf16), ("mstab", (P, T), bf16), ("mptab", (P, T), bf16),
        ("gtab", (P, 30), f32),
    ]
    for name, shape, dt_ in ins:
        io[name] = nc.dram_tensor(name, list(shape), dt_, kind="ExternalInput")
    io["outT"] = nc.dram_tensor("outT", [H, S], bf16, kind="ExternalOutput")

    with tile.TileContext(nc) as tc:
        _emit(nc, tc, io, cfg)
    nc.compile()
    return nc


# ======================================================================
# host side
# ======================================================================

def make_tables(lam_re, lam_im, cfg):
    T = cfg["T"]
    S = cfg["L"] // NCORES
    NSUB = S // T
    f32 = np.float32
    bf = ml_dtypes.bfloat16
    lam = lam_re.astype(np.float64) + 1j * lam_im.astype(np.float64)
    mag = np.abs(lam)
    th = np.angle(lam)
    k = np.arange(T)
    ctab = np.cos(np.outer(th, k))
    stab = np.sin(np.outer(th, k))
    mptab = mag[:, None] ** k[None, :]
    tabs = dict(
        ctab=ctab.astype(bf), stab=stab.astype(bf),
        mctab=(mptab * ctab).astype(bf), mstab=(mptab * stab).astype(bf),
        mptab=mptab.astype(bf),
    )
    tau = np.arange(NSUB)
    lpow = lam[:, None] ** (tau[None, :] * T + 1)
    gcols = dict(
        mvec=mag.astype(f32),
        cosT=np.cos(T * th).astype(f32),
        sinT=np.sin(T * th).astype(f32),
        cosT1=np.cos((T - 1) * th).astype(f32),
        sinT1=np.sin((T - 1) * th).astype(f32),
        lpr=np.real(lpow).astype(f32), lpi=np.imag(lpow).astype(f32),
    )
    coefre = np.zeros((NCORES, P, NCORES), f32)
    coefim = np.zeros((NCORES, P, NCORES), f32)
    for m in range(NCORES):
        for j in range(m):
            v = lam ** (S * (m - 1 - j))
            coefre[m, :, j] = np.real(v)
            coefim[m, :, j] = np.imag(v)
    return tabs, gcols, coefre, coefim


def _block_diag_D(D):
    # (H,) -> (H, 128): row h holds D[h] at column h%128 (per-128 diag blocks)
    Hn = D.shape[0]
    out = np.zeros((Hn, 128), np.float32)
    out[np.arange(Hn), np.arange(Hn) % 128] = D
    return out


def make_in_maps(inputs, cfg):
    f32 = np.float32
    bf = ml_dtypes.bfloat16
    Lc = cfg["L"]
    S = Lc // NCORES
    u = np.ascontiguousarray(np.asarray(inputs["input_sequence"], f32)[:Lc])
    tabs, gcols, coefre, coefim = make_tables(
        np.asarray(inputs["Lambda_re"]), np.asarray(inputs["Lambda_im"]), cfg
    )
    F2 = (np.asarray(inputs["F"], np.float64).T @ np.asarray(inputs["Delta"], np.float64))
    shared = dict(
        BTre=np.ascontiguousarray(np.asarray(inputs["B_re"], f32).T).astype(bf),
        BTim=np.ascontiguousarray(np.asarray(inputs["B_im"], f32).T).astype(bf),
        CreT=np.ascontiguousarray(np.asarray(inputs["C_re"], f32).T).astype(bf),
        nCimT=np.ascontiguousarray(-np.asarray(inputs["C_im"], f32).T).astype(bf),
        F2T=np.ascontiguousarray(F2).astype(bf),
        ET=np.ascontiguousarray(np.asarray(inputs["E"], f32).T).astype(bf),
        Ddiag=np.ascontiguousarray(_block_diag_D(np.asarray(inputs["D"], f32))).astype(bf),
        **tabs,
    )
    in_maps = []
    for m in range(NCORES):
        zcol = np.ones((P, 1), f32)
        if m == 0:
            zcol[:, 0] = 0.0
        gtab = np.concatenate(
            [
                gcols["mvec"][:, None], gcols["cosT"][:, None], gcols["sinT"][:, None],
                gcols["cosT1"][:, None], gcols["sinT1"][:, None],
                coefre[m], coefim[m], zcol,
                gcols["lpr"], gcols["lpi"],
            ],
            axis=1,
        ).astype(f32)
        im = dict(shared)
        im["uT"] = np.ascontiguousarray(u[m * S : (m + 1) * S, :].T).astype(bf)
        im["gtab"] = np.ascontiguousarray(gtab)
        in_maps.append(im)
    return in_maps


def assemble_output(results, cfg):
    Lc = cfg["L"]
    S = Lc // NCORES
    out = np.empty((Lc, H), np.float32)
    for m in range(NCORES):
        out[m * S : (m + 1) * S, :] = results[m]["outT"].T.astype(np.float32)
    out[0, :] = 0.0
    return out


def get_program(cfg_key="full"):
    if cfg_key not in _PROG_CACHE:
        _PROG_CACHE[cfg_key] = build_program(CFG_FULL)
    return _PROG_CACHE[cfg_key]


def run(inputs, trace=False, **kw):
    from concourse import bass_utils

    nc = get_program()
    in_maps = make_in_maps(inputs, CFG_FULL)
    res = bass_utils.run_bass_kernel_spmd(
        nc, in_maps, core_ids=list(range(NCORES)), trace=trace, **kw
    )
    return assemble_output(res.results, CFG_FULL), res


def kernel(**inputs):
    out, _ = run(inputs)
    return out
